# revision 36
# baseline (speedup 1.0000x reference)
"""Trainium2 Bass kernel for nn_AttentionSE3 (graph attention message passing).

Strategy (edge/graph parallel, fully host-prepped ELL layout):
- Attention is a segment softmax over incoming edges of each dst node.  Logits are
  dot(k_edge, q_dst)/sqrt(128) with k,q ~ N(0,1): |logit| <~ 2, so the max-subtraction
  is dropped (softmax is shift-invariant; exp() never overflows here) and
  out[n] = sum_e exp(logit_e) * v_e / sum_e exp(logit_e).
- Host sorts nodes by in-degree, packs them into 128-node blocks, and pads each
  block's per-node edge lists to the block max degree D (rounded to a multiple
  of 4; degree sorting keeps padding small).  Blocks are dealt round-robin to
  the 8 cores; per-group capacity is the max over the 8 cores so EVERY core
  runs the same static program (no collectives: no node's edges span cores).
- Host packs ONE interleaved kv array: per (group, node-row) the D key rows
  [D,128] then the D value rows [D,96] (value columns permuted to [c12,h8]).
  One DMA per group loads both.
- Device program per group-run (consecutive equal-D groups merge into one set
  of wide ops), software-pipelined so no engine's in-order queue waits on
  same-iteration cross-engine results:
  iteration i emits:  k*q multiply(i) (q broadcast over d; a SMALL slice on
  GPSIMD -- heavy concurrent GPSIMD work poisons VectorE's SBUF perf modes);
  VectorE tensor_reduce over contiguous inner k=16 -> logits(i); ScalarE
  exp(i) and the c-expansion ewx(i); then the DEFERRED value side of run i-1
  (multiply, two contiguous d-halvings, strided reduce); then the
  denominator reduce(i) (a second, transposed exp makes it contiguous-inner).
  Padded slots contribute exp(0)=1; a per-node pad count is subtracted
  (exact).  Output accumulates in SBUF; normalization + store run in two
  chunks, the first emitted a few iterations early so the tail overlaps.
  kernel() sets NEURON_RT_RESET_CORES=1: repeated runs otherwise leave the
  cores in a ~15%-degraded-throughput state.
"""

import numpy as np

import concourse.bacc as bacc
import concourse.mybir as mybir
from concourse import tile
from concourse.bass_utils import run_bass_kernel_spmd

try:
    import ml_dtypes
    BF16_NP = np.dtype(ml_dtypes.bfloat16)
except ImportError:  # pragma: no cover
    BF16_NP = None

N_NODES = 50000
H = 8
P = 128  # nodes per block
N_CORES = 8
SCALE = float(1.0 / np.sqrt(128.0))
F32 = mybir.dt.float32
DT_NP = BF16_NP

# engine split knobs.  GPSIMD owns a PRIVATE end-to-end logits pipeline
# (k*q multiply + halving tree) for the last GP_FRAC_PIPE of each run's
# d-range, emitted one run AHEAD so nothing downstream waits on it.
GP_FRAC_PIPE = 0.5    # d-fraction of the logits pipeline owned by GPSIMD
GP_FRAC_VMUL = 0.0    # of the v*ew multiply d-range (c-bcast is slow on gp)
D_ROUND = 4           # capacities rounded up to this multiple
MERGE_CAP = 48        # max merged capacity per run (SBUF-bounded)

# value columns permuted from [h(8), cx(12)] to [cx(12), h(8)] to match the
# ewx expansion layout (c outer, h inner)
PERM_V = np.arange(96).reshape(8, 12).T.reshape(-1)  # new cx*8+h -> old h*12+cx
PERM_V_INV = np.argsort(PERM_V)


# ---------------------------------------------------------------- host prep

def prepare(value, key, query0, query1, edge_index, n_nodes=N_NODES, n_cores=N_CORES):
    """Build per-core padded ELL shards.  Returns (in_maps, meta)."""
    value = np.asarray(value, dtype=np.float32)
    key = np.asarray(key, dtype=np.float32)
    query0 = np.asarray(query0, dtype=np.float32)
    query1 = np.asarray(query1, dtype=np.float32)
    n_edges = key.shape[0]

    dst = np.asarray(edge_index[1], dtype=np.int64)
    deg = np.bincount(dst, minlength=n_nodes).astype(np.int64)
    n_pad = -(-n_nodes // (P * n_cores)) * (P * n_cores)  # round up to 1024
    deg_pad = np.concatenate([deg, np.zeros(n_pad - n_nodes, dtype=np.int64)])
    nb = n_pad // P
    ng = nb // n_cores

    order = np.argsort(deg_pad, kind="stable")  # node ids, degree-ascending
    degs_o = deg_pad[order]

    blk_max = degs_o.reshape(nb, P).max(axis=1)
    D_eff = np.maximum(blk_max.reshape(ng, n_cores).max(axis=1), 1).astype(np.int64)
    D_eff = (D_eff + D_ROUND - 1) // D_ROUND * D_ROUND
    off = np.concatenate([[0], np.cumsum(P * D_eff)]).astype(np.int64)
    S = int(off[-1])  # slots per core

    pos = np.arange(n_pad)
    block = pos // P
    g_of = block // n_cores
    core_of = block % n_cores
    row = pos % P
    Dg = D_eff[g_of]
    base = off[g_of] + row * Dg

    edge_order = np.argsort(dst, kind="stable")
    starts = np.concatenate([[0], np.cumsum(deg)])

    pp = np.repeat(pos, degs_o)
    cum0 = np.concatenate([[0], np.cumsum(degs_o)])[:-1]
    d_idx = np.arange(n_edges) - np.repeat(cum0, degs_o)
    node_of_pp = order[pp]
    edge_ids = edge_order[starts[node_of_pp] + d_idx]
    slot_global = core_of[pp] * S + base[pp] + d_idx

    kp = np.zeros((n_cores * S, 128), dtype=np.float32)
    kp[slot_global] = key[edge_ids]
    vp = np.zeros((n_cores * S, 96), dtype=np.float32)
    vp[slot_global] = value.reshape(n_edges, 96)[:, PERM_V][edge_ids]
    kp = kp.reshape(n_cores, S, 128).astype(DT_NP)
    vp = vp.reshape(n_cores, S, 96).astype(DT_NP)

    qfull = np.concatenate([query0, query1], axis=-1).reshape(n_nodes, 128)
    q_pad = np.zeros((n_pad, 128), dtype=np.float32)
    q_pad[:n_nodes] = qfull
    q_sorted = q_pad[order].reshape(nb, P, 128)

    pc = (Dg - degs_o).astype(np.float32)
    zero_deg = degs_o == 0
    pc[zero_deg] = (Dg[zero_deg] - 1).astype(np.float32)
    pc_sorted = pc.reshape(nb, P)

    runs = merge_runs(D_eff)
    in_maps = []
    for c in range(n_cores):
        # kv packed per RUN: all GM k-blocks [P, GM*D*128] then all GM
        # v-blocks [P, GM*D*96] contiguous -> one DMA + one wide v-multiply
        parts = []
        for g0, GM, D in runs:
            s0, s1 = int(off[g0]), int(off[g0 + GM])
            kb = (kp[c, s0:s1].reshape(GM, P, D * 128)
                  .transpose(1, 0, 2).reshape(-1))
            vb = (vp[c, s0:s1].reshape(GM, P, D * 96)
                  .transpose(1, 0, 2).reshape(-1))
            parts.append(np.concatenate([kb, vb]))
        kv_c = np.concatenate(parts)
        q_c = np.ascontiguousarray(
            q_sorted[c::n_cores].transpose(1, 0, 2).reshape(P, ng * 128)).astype(DT_NP)
        pc_c = np.repeat(np.ascontiguousarray(pc_sorted[c::n_cores].T), H, axis=1)
        in_maps.append({"kv": kv_c, "q": q_c, "pc": pc_c})

    meta = dict(D_eff=D_eff, off=off, S=S, NG=ng, NB=nb, order=order,
                n_nodes=n_nodes, n_pad=n_pad)
    return in_maps, meta


def unshard_output(out_cores, meta):
    """out_cores: list of [128, NG*96] -> [n_nodes, 32, 3]."""
    ng, nb = meta["NG"], meta["NB"]
    n_cores = len(out_cores)
    order, n_nodes, n_pad = meta["order"], meta["n_nodes"], meta["n_pad"]
    out_sorted = np.zeros((nb, P, 96), dtype=np.float32)
    for c in range(n_cores):
        out_sorted[c::n_cores] = (
            out_cores[c].reshape(P, ng, 96).transpose(1, 0, 2))
    out_sorted = out_sorted.reshape(n_pad, 96)[:, PERM_V_INV]
    out_full = np.zeros((n_nodes, 96), dtype=np.float32)
    mask = order < n_nodes
    out_full[order[mask]] = out_sorted[mask]
    return out_full.reshape(n_nodes, 32, 3)


# ---------------------------------------------------------------- bass kernel

def merge_runs(D_eff, cap=MERGE_CAP):
    """[(g0, GM, D), ...]: consecutive equal-D groups fused while GM*D <= cap."""
    runs = []
    g = 0
    ng = len(D_eff)
    while g < ng:
        D = int(D_eff[g])
        gm = 1
        while g + gm < ng and int(D_eff[g + gm]) == D and (gm + 1) * D <= cap:
            gm += 1
        runs.append((g, gm, D))
        g += gm
    return runs


def build(D_eff, S, NG, n_cores=N_CORES):
    D_eff = [int(d) for d in D_eff]
    off = np.concatenate([[0], np.cumsum([P * d for d in D_eff])]).astype(np.int64)

    nc = bacc.Bacc("TRN2", target_bir_lowering=False, debug=False,
                   num_devices=n_cores)
    DT = mybir.dt.bfloat16
    kv = nc.declare_dram_parameter("kv", [S * 224], DT, isOutput=False)
    q = nc.declare_dram_parameter("q", [P, NG * 128], DT, isOutput=False)
    pc = nc.declare_dram_parameter("pc", [P, NG * H], F32, isOutput=False)
    out = nc.declare_dram_parameter("out", [P, NG * 96], F32, isOutput=True)

    mult = mybir.AluOpType.mult
    add = mybir.AluOpType.add
    AX = mybir.AxisListType.X
    runs = merge_runs(D_eff)

    with tile.TileContext(nc) as tc:
        with tc.tile_pool(name="res", bufs=1) as res, \
             tc.tile_pool(name="work3", bufs=3) as work3, \
             tc.tile_pool(name="vtlp", bufs=4) as vtlp, \
             tc.tile_pool(name="wvp", bufs=2) as wvp, \
             tc.tile_pool(name="ewp", bufs=3) as ewp, \
             tc.tile_pool(name="lgp", bufs=3) as lgp, \
             tc.tile_pool(name="gpp", bufs=2) as gpp, \
             tc.tile_pool(name="work", bufs=2) as work, \
             tc.tile_pool(name="small", bufs=2) as small, \
             tc.tile_pool(name="tv", bufs=1) as tvp:
            q_sb = res.tile([P, NG * 128], DT)
            nc.sync.dma_start(q_sb[:], q[:])
            pc_sb = res.tile([P, NG * H], F32)
            nc.sync.dma_start(pc_sb[:], pc[:])
            out_sb = res.tile([P, NG * 96], F32)
            ss_all = res.tile([P, NG * H], F32)

            def value_aps(st):
                g0, GM, D, vtl, ew, ew2, wv = st
                GD = GM * D
                vt = vtl[:].rearrange("n (gd c h) -> n gd c h", c=12, h=H)
                wv4 = wv[:].rearrange("n (gd c h) -> n gd c h", c=12, h=H)
                ewb = (ew[:].rearrange("n (gd h) -> n gd h", h=H)
                       .unsqueeze(2).broadcast_to([P, GD, 12, H]))
                dv = GD - int(round(GD * GP_FRAC_VMUL))
                return vt, wv4, ewb, dv, GD

            def emit_value_gp(st):
                """GPSIMD slice of the previous run's v*ew multiply; emitted
                first each iteration so the gp queue runs well ahead of the
                VectorE halvings that consume it."""
                vt, wv4, ewb, dv, GD = value_aps(st)
                if dv < GD:
                    nc.gpsimd.tensor_tensor(
                        out=wv4[:, dv:], in0=vt[:, dv:],
                        in1=ewb[:, dv:], op=mult)

            def emit_value_vec(st):
                """v*ew multiply (ew broadcast over the 12 c-channels) +
                d-halving chain for a previous run (software pipelining:
                its ew is long done, so nothing stalls)."""
                g0, GM, D, vtl, ew, ew2, wv = st
                vt, wv4, ewb, dv, GD = value_aps(st)
                if dv > 0:
                    nc.vector.tensor_tensor(
                        out=wv4[:, :dv], in0=vt[:, :dv],
                        in1=ewb[:, :dv], op=mult)
                # halve over d while even; finish with a strided reduce (odd
                # d' > 1) or write the last halving straight to out_sb (d'==1)
                out_ap = (out_sb[:, g0 * 96:(g0 + GM) * 96]
                          .rearrange("n (g ch) -> n g ch", g=GM))
                cur, dcur, lvl = wv, D, 0
                while dcur % 2 == 0:
                    dnew = dcur // 2
                    ch = cur[:].rearrange("n (g two dch) -> n g two dch",
                                          g=GM, two=2)
                    if dnew == 1:  # final halving: fp32 straight to out_sb
                        nc.vector.tensor_tensor(
                            out=out_ap, in0=ch[:, :, 0], in1=ch[:, :, 1],
                            op=add)
                        dcur = 1
                        break
                    nxt = tvp.tile([P, GM * dnew * 96], DT,
                                   tag=f"tv{lvl}")
                    nc.vector.tensor_tensor(
                        out=nxt[:].rearrange("n (g dch) -> n g dch", g=GM),
                        in0=ch[:, :, 0], in1=ch[:, :, 1], op=add)
                    cur, dcur, lvl = nxt, dnew, lvl + 1
                if dcur > 1:
                    nc.vector.tensor_reduce(
                        out=out_ap,
                        in_=cur[:].rearrange("n (g d ch) -> n g ch d",
                                             g=GM, ch=96),
                        axis=AX, op=add)

            def emit_denom(st):
                """Deferred denominator: contiguous-inner reduce of the
                transposed exp from two runs back (ScalarE long done)."""
                g0, GM, D, vtl, ew, ew2, wv = st
                nc.vector.tensor_reduce(
                    out=ss_all[:, g0 * H:(g0 + GM) * H],
                    in_=ew2[:].rearrange("n (gh d) -> n gh d", d=D),
                    axis=AX, op=add)

            dn_all = res.tile([P, NG * H], F32)
            rs_all = res.tile([P, NG * H], F32)

            def emit_norm(lo, hi):
                """Normalize + store groups [lo, hi) (out_sb rows final)."""
                nc.vector.tensor_sub(out=dn_all[:, lo * H:hi * H],
                                     in0=ss_all[:, lo * H:hi * H],
                                     in1=pc_sb[:, lo * H:hi * H])
                nc.vector.reciprocal(out=rs_all[:, lo * H:hi * H],
                                     in_=dn_all[:, lo * H:hi * H])
                ngc = hi - lo
                rsb = (rs_all[:, lo * H:hi * H]
                       .rearrange("n (g h) -> n g h", g=ngc)
                       .unsqueeze(2).broadcast_to([P, ngc, 12, H]))
                nc.vector.tensor_tensor(
                    out=(out_sb[:, lo * 96:hi * 96]
                         .rearrange("n (g c h) -> n g c h", g=ngc, c=12)),
                    in0=(out_sb[:, lo * 96:hi * 96]
                         .rearrange("n (g c h) -> n g c h", g=ngc, c=12)),
                    in1=rsb, op=mult)
                nc.sync.dma_start(out[:, lo * 96:hi * 96],
                                  out_sb[:, lo * 96:hi * 96])

            # element offset of each run in flat kv
            rbases = [0]
            for g0, GM, D in runs:
                rbases.append(rbases[-1] + P * GM * D * 224)

            def dma_run(i):
                """Issue the k/v/q DMAs for run i; returns (ktl, vtl)."""
                g0, GM, D = runs[i]
                GD = GM * D
                rb = rbases[i]
                ktl = work3.tile([P, GD * 128], DT, tag="ktl")
                nc.sync.dma_start(
                    ktl[:],
                    kv[rb:rb + P * GD * 128].rearrange("(n w) -> n w", n=P))
                vtl = vtlp.tile([P, GD * 96], DT, tag="vtl")
                nc.sync.dma_start(
                    vtl[:],
                    kv[rb + P * GD * 128:rb + P * GD * 224]
                    .rearrange("(n w) -> n w", n=P))
                return ktl, vtl

            def emit_logit_chain(eng, wt, gdh_n, lg_ap, pool, pfx):
                """k*q product tile wt [P, gdh_n*16] -> logits lg_ap
                [.., gdh_n] via a halving tree of tensor_tensor adds (2x
                bf16 mode; tensor_reduce is capped at 1x).  Pairs are
                (i, i+half) within each slot's 16-feature run."""
                wa = pool.tile([P, gdh_n * 8], DT, tag=pfx + "a")
                wh = wt[:].rearrange("n (gdh two k) -> n gdh two k",
                                     two=2, k=8)
                eng.tensor_tensor(
                    out=wa[:].rearrange("n (gdh k) -> n gdh k", k=8),
                    in0=wh[:, :, 0], in1=wh[:, :, 1], op=add)
                wb = pool.tile([P, gdh_n * 4], DT, tag=pfx + "b")
                wah = wa[:].rearrange("n (gdh two k) -> n gdh two k",
                                      two=2, k=4)
                eng.tensor_tensor(
                    out=wb[:].rearrange("n (gdh k) -> n gdh k", k=4),
                    in0=wah[:, :, 0], in1=wah[:, :, 1], op=add)
                wc = pool.tile([P, gdh_n * 2], DT, tag=pfx + "c")
                wbh = wb[:].rearrange("n (gdh two k) -> n gdh two k",
                                      two=2, k=2)
                eng.tensor_tensor(
                    out=wc[:].rearrange("n (gdh k) -> n gdh k", k=2),
                    in0=wbh[:, :, 0], in1=wbh[:, :, 1], op=add)
                wch = wc[:].rearrange("n (gdh two) -> n gdh two", two=2)
                eng.tensor_tensor(
                    out=lg_ap, in0=wch[:, :, 0], in1=wch[:, :, 1], op=add)

            def qb_of(g0, GM, D):
                return (q_sb[:, g0 * 128:(g0 + GM) * 128]
                        .rearrange("n (g f) -> n g f", g=GM)
                        .unsqueeze(2).broadcast_to([P, GM, D, 128]))

            def dsplit(D):
                dgp = int(round(D * GP_FRAC_PIPE))
                return D - dgp, dgp

            def emit_gp_chain(j, ktl_j):
                """GPSIMD's private logits pipeline for run j (emitted one
                run ahead): kq multiply + halving tree on the d-range
                [dv, D), writing its slice of lg(j)."""
                g0, GM, D = runs[j]
                dv, dgp = dsplit(D)
                lg = lgp.tile([P, GM * D * H], F32, tag="lg")
                if dgp > 0:
                    kt = ktl_j[:].rearrange("n (g d f) -> n g d f",
                                            g=GM, f=128)
                    gw = gpp.tile([P, GM * dgp * 128], DT, tag="gw")
                    nc.gpsimd.tensor_tensor(
                        out=gw[:].rearrange("n (g d f) -> n g d f",
                                            g=GM, f=128),
                        in0=kt[:, :, dv:], in1=qb_of(g0, GM, D)[:, :, dv:],
                        op=mult)
                    lg_gp = (lg[:].rearrange("n (g d h) -> n g d h",
                                             g=GM, h=H)[:, :, dv:]
                             .rearrange("n g d h -> n g (d h)"))
                    emit_logit_chain(nc.gpsimd, gw, GM * dgp * H,
                                     lg_gp, gpp, "g")
                return lg

            # software pipeline: the value side is deferred TWO runs and the
            # GPSIMD logit chain runs ONE run ahead, so every cross-engine
            # input of an emitted op is >= 1 full phase old
            split_i = max(2, len(runs) - 3)
            p1 = p2 = None  # runs i-1 / i-2 state
            tiles = dma_run(0)
            lg_next = emit_gp_chain(0, tiles[0])
            for i, (g0, GM, D) in enumerate(runs):
                ktl, vtl = tiles
                lg = lg_next
                if i + 1 < len(runs):
                    tiles = dma_run(i + 1)
                    lg_next = emit_gp_chain(i + 1, tiles[0])

                kt = ktl[:].rearrange("n (g d f) -> n g d f", g=GM, f=128)
                dv, dgp = dsplit(D)

                # VectorE: deferred value side + denominator of run i-2
                if p2 is not None:
                    wv = wvp.tile([P, p2[1] * p2[2] * 96], DT, tag="wv")
                    p2 = p2[:6] + (wv,)
                    emit_value_gp(p2)
                    emit_value_vec(p2)
                    emit_denom(p2)

                if i == split_i:
                    # denominators complete for all runs < split_i - 1
                    emit_norm(0, runs[split_i - 1][0])

                # VectorE's share of the logits pipeline: d-range [0, dv)
                if dv > 0:
                    w = work.tile([P, GM * dv * 128], DT, tag="w")
                    nc.vector.tensor_tensor(
                        out=w[:].rearrange("n (g d f) -> n g d f",
                                           g=GM, f=128),
                        in0=kt[:, :, :dv], in1=qb_of(g0, GM, D)[:, :, :dv],
                        op=mult)
                    lg_v = (lg[:].rearrange("n (g d h) -> n g d h",
                                            g=GM, h=H)[:, :, :dv]
                            .rearrange("n g d h -> n g (d h)"))
                    emit_logit_chain(nc.vector, w, GM * dv * H,
                                     lg_v, small, "v")

                # ew = exp(scale * logits)  (contiguous [g, d, h])
                ew = ewp.tile([P, GM * D * H], DT, tag="ew")
                nc.scalar.activation(
                    out=ew[:], in_=lg[:],
                    func=mybir.ActivationFunctionType.Exp, scale=SCALE)
                # second exp transposed to [g, h, d]: the denominator reduce
                # becomes contiguous-inner
                ew2 = ewp.tile([P, GM * D * H], DT, tag="ew2")
                nc.scalar.activation(
                    out=ew2[:].rearrange("n (g h d) -> n g d h", g=GM, h=H),
                    in_=lg[:].rearrange("n (g d h) -> n g d h", g=GM, h=H),
                    func=mybir.ActivationFunctionType.Exp, scale=SCALE)

                p1, p2 = (g0, GM, D, vtl, ew, ew2, None), p1

            for st in (p2, p1):  # flush the two pending value sides
                if st is None:
                    continue
                wv = wvp.tile([P, st[1] * st[2] * 96], DT, tag="wv")
                st = st[:6] + (wv,)
                emit_value_gp(st)
                emit_value_vec(st)
                emit_denom(st)
            emit_norm(runs[split_i - 1][0], NG)

    nc.compile()
    return nc


# ---------------------------------------------------------------- entry point

LAST_RESULT = None  # BassKernelResults of the most recent run (for test harness)


def kernel(value, key, query0, query1, edge_index):
    global LAST_RESULT
    import os
    # repeated runs leave the NeuronCores in a degraded-throughput state
    # (~15% slower engines); a core reset at runtime init restores it
    os.environ.setdefault("NEURON_RT_RESET_CORES", "1")
    in_maps, meta = prepare(value, key, query0, query1, edge_index)
    nc = build(meta["D_eff"], meta["S"], meta["NG"])
    res = run_bass_kernel_spmd(nc, in_maps, list(range(N_CORES)),
                               tmpdir=os.environ.get("BASS_SPMD_TMPDIR"))
    LAST_RESULT = res
    out_cores = [res.results[c]["out"] for c in range(N_CORES)]
    return unshard_output(out_cores, meta)



# revision 37
# speedup vs baseline: 1.2164x; 1.2164x over previous
"""Trainium2 Bass kernel for nn_AttentionSE3 (graph attention message passing).

Strategy (edge/graph parallel, fully host-prepped ELL layout):
- Attention is a segment softmax over incoming edges of each dst node.  Logits are
  dot(k_edge, q_dst)/sqrt(128) with k,q ~ N(0,1): |logit| <~ 2, so the max-subtraction
  is dropped (softmax is shift-invariant; exp() never overflows here) and
  out[n] = sum_e exp(logit_e) * v_e / sum_e exp(logit_e).
- Host sorts nodes by in-degree, packs them into 128-node blocks, and pads each
  block's per-node edge lists to the block max degree D (rounded to a multiple
  of 4; degree sorting keeps padding small).  Blocks are dealt round-robin to
  the 8 cores; per-group capacity is the max over the 8 cores so EVERY core
  runs the same static program (no collectives: no node's edges span cores).
- Host packs ONE interleaved kv array: per (group, node-row) the D key rows
  [D,128] then the D value rows [D,96] (value columns permuted to [c12,h8]).
  One DMA per group loads both.
- Device program per group-run (consecutive equal-D groups merge into one set
  of wide ops), software-pipelined so no engine's in-order queue waits on
  same-iteration cross-engine results:
  iteration i emits:  k*q multiply(i) (q broadcast over d; a SMALL slice on
  GPSIMD -- heavy concurrent GPSIMD work poisons VectorE's SBUF perf modes);
  VectorE tensor_reduce over contiguous inner k=16 -> logits(i); ScalarE
  exp(i) and the c-expansion ewx(i); then the DEFERRED value side of run i-1
  (multiply, two contiguous d-halvings, strided reduce); then the
  denominator reduce(i) (a second, transposed exp makes it contiguous-inner).
  Padded slots contribute exp(0)=1; a per-node pad count is subtracted
  (exact).  Output accumulates in SBUF; normalization + store run in two
  chunks, the first emitted a few iterations early so the tail overlaps.
  kernel() sets NEURON_RT_RESET_CORES=1: repeated runs otherwise leave the
  cores in a ~15%-degraded-throughput state.
"""

import numpy as np

import concourse.bacc as bacc
import concourse.mybir as mybir
from concourse import tile
from concourse.bass_utils import run_bass_kernel_spmd

try:
    import ml_dtypes
    BF16_NP = np.dtype(ml_dtypes.bfloat16)
except ImportError:  # pragma: no cover
    BF16_NP = None

N_NODES = 50000
H = 8
P = 128  # nodes per block
N_CORES = 8
SCALE = float(1.0 / np.sqrt(128.0))
F32 = mybir.dt.float32
DT_NP = BF16_NP

# engine split knobs.  GPSIMD owns a PRIVATE end-to-end logits pipeline
# (k*q multiply + halving tree) for the last GP_FRAC_PIPE of each run's
# d-range, emitted one run AHEAD so nothing downstream waits on it.
GP_FRAC_PIPE = 0.25    # d-fraction of the logits pipeline owned by GPSIMD
GP_FRAC_VMUL = 0.0    # of the v*ew multiply d-range (c-bcast is slow on gp)
D_ROUND = 4           # capacities rounded up to this multiple
MERGE_CAP = 48        # max merged capacity per run (SBUF-bounded)

# value columns permuted from [h(8), cx(12)] to [cx(12), h(8)] to match the
# ewx expansion layout (c outer, h inner)
PERM_V = np.arange(96).reshape(8, 12).T.reshape(-1)  # new cx*8+h -> old h*12+cx
PERM_V_INV = np.argsort(PERM_V)


# ---------------------------------------------------------------- host prep

def prepare(value, key, query0, query1, edge_index, n_nodes=N_NODES, n_cores=N_CORES):
    """Build per-core padded ELL shards.  Returns (in_maps, meta)."""
    value = np.asarray(value, dtype=np.float32)
    key = np.asarray(key, dtype=np.float32)
    query0 = np.asarray(query0, dtype=np.float32)
    query1 = np.asarray(query1, dtype=np.float32)
    n_edges = key.shape[0]

    dst = np.asarray(edge_index[1], dtype=np.int64)
    deg = np.bincount(dst, minlength=n_nodes).astype(np.int64)
    n_pad = -(-n_nodes // (P * n_cores)) * (P * n_cores)  # round up to 1024
    deg_pad = np.concatenate([deg, np.zeros(n_pad - n_nodes, dtype=np.int64)])
    nb = n_pad // P
    ng = nb // n_cores

    order = np.argsort(deg_pad, kind="stable")  # node ids, degree-ascending
    degs_o = deg_pad[order]

    blk_max = degs_o.reshape(nb, P).max(axis=1)
    D_eff = np.maximum(blk_max.reshape(ng, n_cores).max(axis=1), 1).astype(np.int64)
    D_eff = (D_eff + D_ROUND - 1) // D_ROUND * D_ROUND
    off = np.concatenate([[0], np.cumsum(P * D_eff)]).astype(np.int64)
    S = int(off[-1])  # slots per core

    pos = np.arange(n_pad)
    block = pos // P
    g_of = block // n_cores
    core_of = block % n_cores
    row = pos % P
    Dg = D_eff[g_of]
    base = off[g_of] + row * Dg

    edge_order = np.argsort(dst, kind="stable")
    starts = np.concatenate([[0], np.cumsum(deg)])

    pp = np.repeat(pos, degs_o)
    cum0 = np.concatenate([[0], np.cumsum(degs_o)])[:-1]
    d_idx = np.arange(n_edges) - np.repeat(cum0, degs_o)
    node_of_pp = order[pp]
    edge_ids = edge_order[starts[node_of_pp] + d_idx]
    slot_global = core_of[pp] * S + base[pp] + d_idx

    kp = np.zeros((n_cores * S, 128), dtype=np.float32)
    kp[slot_global] = key[edge_ids]
    vp = np.zeros((n_cores * S, 96), dtype=np.float32)
    vp[slot_global] = value.reshape(n_edges, 96)[:, PERM_V][edge_ids]
    kp = kp.reshape(n_cores, S, 128).astype(DT_NP)
    vp = vp.reshape(n_cores, S, 96).astype(DT_NP)

    qfull = np.concatenate([query0, query1], axis=-1).reshape(n_nodes, 128)
    q_pad = np.zeros((n_pad, 128), dtype=np.float32)
    q_pad[:n_nodes] = qfull
    q_sorted = q_pad[order].reshape(nb, P, 128)

    pc = (Dg - degs_o).astype(np.float32)
    zero_deg = degs_o == 0
    pc[zero_deg] = (Dg[zero_deg] - 1).astype(np.float32)
    pc_sorted = pc.reshape(nb, P)

    runs = merge_runs(D_eff)
    in_maps = []
    for c in range(n_cores):
        # kv packed per RUN: all GM k-blocks [P, GM*D*128] then all GM
        # v-blocks [P, GM*D*96] contiguous -> one DMA + one wide v-multiply
        parts = []
        for g0, GM, D in runs:
            s0, s1 = int(off[g0]), int(off[g0 + GM])
            kb = (kp[c, s0:s1].reshape(GM, P, D * 128)
                  .transpose(1, 0, 2).reshape(-1))
            vb = (vp[c, s0:s1].reshape(GM, P, D * 96)
                  .transpose(1, 0, 2).reshape(-1))
            parts.append(np.concatenate([kb, vb]))
        kv_c = np.concatenate(parts)
        q_c = np.ascontiguousarray(
            q_sorted[c::n_cores].transpose(1, 0, 2).reshape(P, ng * 128)).astype(DT_NP)
        pc_c = np.repeat(np.ascontiguousarray(pc_sorted[c::n_cores].T), H, axis=1)
        in_maps.append({"kv": kv_c, "q": q_c, "pc": pc_c})

    meta = dict(D_eff=D_eff, off=off, S=S, NG=ng, NB=nb, order=order,
                n_nodes=n_nodes, n_pad=n_pad)
    return in_maps, meta


def unshard_output(out_cores, meta):
    """out_cores: list of [128, NG*96] -> [n_nodes, 32, 3]."""
    ng, nb = meta["NG"], meta["NB"]
    n_cores = len(out_cores)
    order, n_nodes, n_pad = meta["order"], meta["n_nodes"], meta["n_pad"]
    out_sorted = np.zeros((nb, P, 96), dtype=np.float32)
    for c in range(n_cores):
        out_sorted[c::n_cores] = (
            out_cores[c].reshape(P, ng, 96).transpose(1, 0, 2))
    out_sorted = out_sorted.reshape(n_pad, 96)[:, PERM_V_INV]
    out_full = np.zeros((n_nodes, 96), dtype=np.float32)
    mask = order < n_nodes
    out_full[order[mask]] = out_sorted[mask]
    return out_full.reshape(n_nodes, 32, 3)


# ---------------------------------------------------------------- bass kernel

def merge_runs(D_eff, cap=MERGE_CAP):
    """[(g0, GM, D), ...]: consecutive equal-D groups fused while GM*D <= cap."""
    runs = []
    g = 0
    ng = len(D_eff)
    while g < ng:
        D = int(D_eff[g])
        gm = 1
        while g + gm < ng and int(D_eff[g + gm]) == D and (gm + 1) * D <= cap:
            gm += 1
        runs.append((g, gm, D))
        g += gm
    return runs


def build(D_eff, S, NG, n_cores=N_CORES):
    D_eff = [int(d) for d in D_eff]
    off = np.concatenate([[0], np.cumsum([P * d for d in D_eff])]).astype(np.int64)

    nc = bacc.Bacc("TRN2", target_bir_lowering=False, debug=False,
                   num_devices=n_cores)
    DT = mybir.dt.bfloat16
    kv = nc.declare_dram_parameter("kv", [S * 224], DT, isOutput=False)
    q = nc.declare_dram_parameter("q", [P, NG * 128], DT, isOutput=False)
    pc = nc.declare_dram_parameter("pc", [P, NG * H], F32, isOutput=False)
    out = nc.declare_dram_parameter("out", [P, NG * 96], F32, isOutput=True)

    mult = mybir.AluOpType.mult
    add = mybir.AluOpType.add
    AX = mybir.AxisListType.X
    runs = merge_runs(D_eff)

    with tile.TileContext(nc) as tc:
        with tc.tile_pool(name="res", bufs=1) as res, \
             tc.tile_pool(name="work3", bufs=3) as work3, \
             tc.tile_pool(name="vtlp", bufs=4) as vtlp, \
             tc.tile_pool(name="wvp", bufs=2) as wvp, \
             tc.tile_pool(name="ewp", bufs=3) as ewp, \
             tc.tile_pool(name="lgp", bufs=3) as lgp, \
             tc.tile_pool(name="gpp", bufs=2) as gpp, \
             tc.tile_pool(name="work", bufs=2) as work, \
             tc.tile_pool(name="small", bufs=2) as small, \
             tc.tile_pool(name="tv", bufs=1) as tvp:
            q_sb = res.tile([P, NG * 128], DT)
            nc.sync.dma_start(q_sb[:], q[:])
            pc_sb = res.tile([P, NG * H], F32)
            nc.sync.dma_start(pc_sb[:], pc[:])
            out_sb = res.tile([P, NG * 96], F32)
            ss_all = res.tile([P, NG * H], F32)

            def value_aps(st):
                g0, GM, D, vtl, ew, ew2, wv = st
                GD = GM * D
                vt = vtl[:].rearrange("n (gd c h) -> n gd c h", c=12, h=H)
                wv4 = wv[:].rearrange("n (gd c h) -> n gd c h", c=12, h=H)
                ewb = (ew[:].rearrange("n (gd h) -> n gd h", h=H)
                       .unsqueeze(2).broadcast_to([P, GD, 12, H]))
                dv = GD - int(round(GD * GP_FRAC_VMUL))
                return vt, wv4, ewb, dv, GD

            def emit_value_gp(st):
                """GPSIMD slice of the previous run's v*ew multiply; emitted
                first each iteration so the gp queue runs well ahead of the
                VectorE halvings that consume it."""
                vt, wv4, ewb, dv, GD = value_aps(st)
                if dv < GD:
                    nc.gpsimd.tensor_tensor(
                        out=wv4[:, dv:], in0=vt[:, dv:],
                        in1=ewb[:, dv:], op=mult)

            def emit_value_vec(st):
                """v*ew multiply (ew broadcast over the 12 c-channels) +
                d-halving chain for a previous run (software pipelining:
                its ew is long done, so nothing stalls)."""
                g0, GM, D, vtl, ew, ew2, wv = st
                vt, wv4, ewb, dv, GD = value_aps(st)
                if dv > 0:
                    nc.vector.tensor_tensor(
                        out=wv4[:, :dv], in0=vt[:, :dv],
                        in1=ewb[:, :dv], op=mult)
                # halve over d while even; finish with a strided reduce (odd
                # d' > 1) or write the last halving straight to out_sb (d'==1)
                out_ap = (out_sb[:, g0 * 96:(g0 + GM) * 96]
                          .rearrange("n (g ch) -> n g ch", g=GM))
                cur, dcur, lvl = wv, D, 0
                while dcur % 2 == 0:
                    dnew = dcur // 2
                    ch = cur[:].rearrange("n (g two dch) -> n g two dch",
                                          g=GM, two=2)
                    if dnew == 1:  # final halving: fp32 straight to out_sb
                        nc.vector.tensor_tensor(
                            out=out_ap, in0=ch[:, :, 0], in1=ch[:, :, 1],
                            op=add)
                        dcur = 1
                        break
                    nxt = tvp.tile([P, GM * dnew * 96], DT,
                                   tag=f"tv{lvl}")
                    nc.vector.tensor_tensor(
                        out=nxt[:].rearrange("n (g dch) -> n g dch", g=GM),
                        in0=ch[:, :, 0], in1=ch[:, :, 1], op=add)
                    cur, dcur, lvl = nxt, dnew, lvl + 1
                if dcur > 1:
                    nc.vector.tensor_reduce(
                        out=out_ap,
                        in_=cur[:].rearrange("n (g d ch) -> n g ch d",
                                             g=GM, ch=96),
                        axis=AX, op=add)

            def emit_denom(st):
                """Deferred denominator: contiguous-inner reduce of the
                transposed exp from two runs back (ScalarE long done)."""
                g0, GM, D, vtl, ew, ew2, wv = st
                nc.vector.tensor_reduce(
                    out=ss_all[:, g0 * H:(g0 + GM) * H],
                    in_=ew2[:].rearrange("n (gh d) -> n gh d", d=D),
                    axis=AX, op=add)

            dn_all = res.tile([P, NG * H], F32)
            rs_all = res.tile([P, NG * H], F32)

            def emit_norm(lo, hi):
                """Normalize + store groups [lo, hi) (out_sb rows final)."""
                nc.vector.tensor_sub(out=dn_all[:, lo * H:hi * H],
                                     in0=ss_all[:, lo * H:hi * H],
                                     in1=pc_sb[:, lo * H:hi * H])
                nc.vector.reciprocal(out=rs_all[:, lo * H:hi * H],
                                     in_=dn_all[:, lo * H:hi * H])
                ngc = hi - lo
                rsb = (rs_all[:, lo * H:hi * H]
                       .rearrange("n (g h) -> n g h", g=ngc)
                       .unsqueeze(2).broadcast_to([P, ngc, 12, H]))
                nc.vector.tensor_tensor(
                    out=(out_sb[:, lo * 96:hi * 96]
                         .rearrange("n (g c h) -> n g c h", g=ngc, c=12)),
                    in0=(out_sb[:, lo * 96:hi * 96]
                         .rearrange("n (g c h) -> n g c h", g=ngc, c=12)),
                    in1=rsb, op=mult)
                nc.sync.dma_start(out[:, lo * 96:hi * 96],
                                  out_sb[:, lo * 96:hi * 96])

            # element offset of each run in flat kv
            rbases = [0]
            for g0, GM, D in runs:
                rbases.append(rbases[-1] + P * GM * D * 224)

            def dma_run(i):
                """Issue the k/v/q DMAs for run i; returns (ktl, vtl)."""
                g0, GM, D = runs[i]
                GD = GM * D
                rb = rbases[i]
                ktl = work3.tile([P, GD * 128], DT, tag="ktl")
                nc.sync.dma_start(
                    ktl[:],
                    kv[rb:rb + P * GD * 128].rearrange("(n w) -> n w", n=P))
                vtl = vtlp.tile([P, GD * 96], DT, tag="vtl")
                nc.sync.dma_start(
                    vtl[:],
                    kv[rb + P * GD * 128:rb + P * GD * 224]
                    .rearrange("(n w) -> n w", n=P))
                return ktl, vtl

            def emit_logit_chain(eng, wt, gdh_n, lg_ap, pool, pfx):
                """k*q product tile wt [P, gdh_n*16] -> logits lg_ap
                [.., gdh_n] via a halving tree of tensor_tensor adds (2x
                bf16 mode; tensor_reduce is capped at 1x).  Pairs are
                (i, i+half) within each slot's 16-feature run."""
                wa = pool.tile([P, gdh_n * 8], DT, tag=pfx + "a")
                wh = wt[:].rearrange("n (gdh two k) -> n gdh two k",
                                     two=2, k=8)
                eng.tensor_tensor(
                    out=wa[:].rearrange("n (gdh k) -> n gdh k", k=8),
                    in0=wh[:, :, 0], in1=wh[:, :, 1], op=add)
                wb = pool.tile([P, gdh_n * 4], DT, tag=pfx + "b")
                wah = wa[:].rearrange("n (gdh two k) -> n gdh two k",
                                      two=2, k=4)
                eng.tensor_tensor(
                    out=wb[:].rearrange("n (gdh k) -> n gdh k", k=4),
                    in0=wah[:, :, 0], in1=wah[:, :, 1], op=add)
                wc = pool.tile([P, gdh_n * 2], DT, tag=pfx + "c")
                wbh = wb[:].rearrange("n (gdh two k) -> n gdh two k",
                                      two=2, k=2)
                eng.tensor_tensor(
                    out=wc[:].rearrange("n (gdh k) -> n gdh k", k=2),
                    in0=wbh[:, :, 0], in1=wbh[:, :, 1], op=add)
                wch = wc[:].rearrange("n (gdh two) -> n gdh two", two=2)
                eng.tensor_tensor(
                    out=lg_ap, in0=wch[:, :, 0], in1=wch[:, :, 1], op=add)

            def qb_of(g0, GM, D):
                return (q_sb[:, g0 * 128:(g0 + GM) * 128]
                        .rearrange("n (g f) -> n g f", g=GM)
                        .unsqueeze(2).broadcast_to([P, GM, D, 128]))

            def dsplit(D):
                dgp = int(round(D * GP_FRAC_PIPE))
                return D - dgp, dgp

            def emit_gp_chain(j, ktl_j):
                """GPSIMD's private logits pipeline for run j (emitted one
                run ahead): kq multiply + halving tree on the d-range
                [dv, D), writing its slice of lg(j)."""
                g0, GM, D = runs[j]
                dv, dgp = dsplit(D)
                lg = lgp.tile([P, GM * D * H], F32, tag="lg")
                if dgp > 0:
                    kt = ktl_j[:].rearrange("n (g d f) -> n g d f",
                                            g=GM, f=128)
                    gw = gpp.tile([P, GM * dgp * 128], DT, tag="gw")
                    nc.gpsimd.tensor_tensor(
                        out=gw[:].rearrange("n (g d f) -> n g d f",
                                            g=GM, f=128),
                        in0=kt[:, :, dv:], in1=qb_of(g0, GM, D)[:, :, dv:],
                        op=mult)
                    lg_gp = (lg[:].rearrange("n (g d h) -> n g d h",
                                             g=GM, h=H)[:, :, dv:]
                             .rearrange("n g d h -> n g (d h)"))
                    emit_logit_chain(nc.gpsimd, gw, GM * dgp * H,
                                     lg_gp, gpp, "g")
                return lg

            # software pipeline: the value side is deferred TWO runs and the
            # GPSIMD logit chain runs ONE run ahead, so every cross-engine
            # input of an emitted op is >= 1 full phase old
            split_i = max(2, len(runs) - 3)
            p1 = p2 = None  # runs i-1 / i-2 state
            tiles = dma_run(0)
            lg_next = emit_gp_chain(0, tiles[0])
            for i, (g0, GM, D) in enumerate(runs):
                ktl, vtl = tiles
                lg = lg_next
                if i + 1 < len(runs):
                    tiles = dma_run(i + 1)
                    lg_next = emit_gp_chain(i + 1, tiles[0])

                kt = ktl[:].rearrange("n (g d f) -> n g d f", g=GM, f=128)
                dv, dgp = dsplit(D)

                # VectorE: deferred value side + denominator of run i-2
                if p2 is not None:
                    wv = wvp.tile([P, p2[1] * p2[2] * 96], DT, tag="wv")
                    p2 = p2[:6] + (wv,)
                    emit_value_gp(p2)
                    emit_value_vec(p2)
                    emit_denom(p2)

                if i == split_i:
                    # denominators complete for all runs < split_i - 1
                    emit_norm(0, runs[split_i - 1][0])

                # VectorE's share of the logits pipeline: d-range [0, dv)
                if dv > 0:
                    w = work.tile([P, GM * dv * 128], DT, tag="w")
                    nc.vector.tensor_tensor(
                        out=w[:].rearrange("n (g d f) -> n g d f",
                                           g=GM, f=128),
                        in0=kt[:, :, :dv], in1=qb_of(g0, GM, D)[:, :, :dv],
                        op=mult)
                    lg_v = (lg[:].rearrange("n (g d h) -> n g d h",
                                            g=GM, h=H)[:, :, :dv]
                            .rearrange("n g d h -> n g (d h)"))
                    emit_logit_chain(nc.vector, w, GM * dv * H,
                                     lg_v, small, "v")

                # ew = exp(scale * logits)  (contiguous [g, d, h])
                ew = ewp.tile([P, GM * D * H], DT, tag="ew")
                nc.scalar.activation(
                    out=ew[:], in_=lg[:],
                    func=mybir.ActivationFunctionType.Exp, scale=SCALE)
                # second exp transposed to [g, h, d]: the denominator reduce
                # becomes contiguous-inner
                ew2 = ewp.tile([P, GM * D * H], DT, tag="ew2")
                nc.scalar.activation(
                    out=ew2[:].rearrange("n (g h d) -> n g d h", g=GM, h=H),
                    in_=lg[:].rearrange("n (g d h) -> n g d h", g=GM, h=H),
                    func=mybir.ActivationFunctionType.Exp, scale=SCALE)

                p1, p2 = (g0, GM, D, vtl, ew, ew2, None), p1

            for st in (p2, p1):  # flush the two pending value sides
                if st is None:
                    continue
                wv = wvp.tile([P, st[1] * st[2] * 96], DT, tag="wv")
                st = st[:6] + (wv,)
                emit_value_gp(st)
                emit_value_vec(st)
                emit_denom(st)
            emit_norm(runs[split_i - 1][0], NG)

    nc.compile()
    return nc


# ---------------------------------------------------------------- entry point

LAST_RESULT = None  # BassKernelResults of the most recent run (for test harness)


def kernel(value, key, query0, query1, edge_index):
    global LAST_RESULT
    import os
    # repeated runs leave the NeuronCores in a degraded-throughput state
    # (~15% slower engines); a core reset at runtime init restores it
    os.environ.setdefault("NEURON_RT_RESET_CORES", "1")
    in_maps, meta = prepare(value, key, query0, query1, edge_index)
    nc = build(meta["D_eff"], meta["S"], meta["NG"])
    res = run_bass_kernel_spmd(nc, in_maps, list(range(N_CORES)),
                               tmpdir=os.environ.get("BASS_SPMD_TMPDIR"))
    LAST_RESULT = res
    out_cores = [res.results[c]["out"] for c in range(N_CORES)]
    return unshard_output(out_cores, meta)

